# revision 9
# baseline (speedup 1.0000x reference)
"""Trainium2 Bass kernel for EntropySamplLoss.

Reference semantics (per image b):
  acts [N, P=320] viewed as [N, S=4, C=8, K=10] prototype groups
  ent[n, s, c] = normalized softmax entropy over the K protos of group (s, c)
  num[s, c]   = sum over pixels n with label c of ent[n, s, c]
  cnt[c]      = number of pixels with label c
  loss = mean over present (b, s, c) of num[s, c] / cnt[c]

Device kernel (data-parallel, one image per NeuronCore):
  per chunk of 1024 pixels (tile [128 part, 2560], 8 px per partition):
    E   = exp(x)                       (ACT)
    Z   = group-sum_k E                (DVE grouped tensor_reduce)
    xE  = x * E                        (GPSIMD)
    U   = group-sum_k xE               (DVE grouped tensor_reduce)
    logZ = ln(Z); rZ = exp(-logZ)      (ACT)
    ent_raw = logZ - U*rZ              (DVE)  [= ln(K) * normalized entropy]
    mask[px, (j,c)] = labels==c+1      (DVE is_equal vs broadcast iota)
    stats[ (j,c), (j',sc|ones) ] += mask^T @ [ent|1]   (PE matmul, PSUM accum)
  host: extract diagonal j==j', divide by ln(K), per-class means, final mean.
"""

import sys

if "/opt/trn_rl_repo" not in sys.path:
    sys.path.insert(0, "/opt/trn_rl_repo")

from contextlib import ExitStack

import numpy as np

import concourse.bacc as bacc
import concourse.bass as bass
import concourse.tile as tile
from concourse import mybir
from concourse.bass_utils import run_bass_kernel_spmd
from concourse.tile import add_dep_helper

# Problem shape (hardcoded per spec)
B, N, PP = 8, 65536, 320
S, C, K = 4, 8, 10
NCORES = 8

PX_PER_PART = 8          # pixels per partition ("j" slots)
PART = 128
PX_PER_CHUNK = PART * PX_PER_PART      # 1024
NCHUNK = N // PX_PER_CHUNK             # 64
FREE = PX_PER_PART * PP                # 2560
G = S * C                              # 32 groups per pixel
GF = PX_PER_PART * G                   # 256 group slots per partition
EW = G + 1                             # 33: ent cols + ones col

_CACHE = {}


def _patch_act_tables():
    """Make the combined exp+ln table set the only candidate for Exp/Ln so
    the table-load placement pass doesn't thrash between per-function sets
    (one ACT_TABLE_LOAD total instead of 2 per chunk)."""
    import concourse.hw_specs as hw_specs

    tabs = hw_specs.get_activation_tables("gen3")
    E = mybir.ActivationFunctionType.Exp
    L = mybir.ActivationFunctionType.Ln
    for name, funcs in tabs.items():
        if name != "natural_log_exp_and_others":
            funcs.discard(E)
            funcs.discard(L)


def _build():
    if "nc" in _CACHE:
        return _CACHE["nc"]

    _patch_act_tables()
    f32 = mybir.dt.float32
    nc = bacc.Bacc("TRN2", target_bir_lowering=False, debug=False, num_devices=NCORES)

    acts = nc.dram_tensor("acts", [NCHUNK, PART, FREE], f32, kind="ExternalInput").ap()
    labels = nc.dram_tensor(
        "labels", [NCHUNK, PART, PX_PER_PART], f32, kind="ExternalInput"
    ).ap()
    consts = nc.dram_tensor("consts", [C + 1], f32, kind="ExternalInput")
    stats_out = nc.dram_tensor(
        "stats", [PX_PER_PART * C, PX_PER_PART * EW], f32, kind="ExternalOutput"
    ).ap()

    with tile.TileContext(nc) as tc:
        with ExitStack() as ctx:
            singles = ctx.enter_context(tc.tile_pool(name="singles", bufs=1))
            big = ctx.enter_context(tc.tile_pool(name="big", bufs=3))
            ebuf = ctx.enter_context(tc.tile_pool(name="ebuf", bufs=2))
            xebuf = ctx.enter_context(tc.tile_pool(name="xebuf", bufs=2))
            small = ctx.enter_context(tc.tile_pool(name="small", bufs=3))
            psum = ctx.enter_context(tc.tile_pool(name="psum", bufs=1, space="PSUM"))

            # constants: [1..8, 1.0] broadcast to all partitions
            cvec = singles.tile([PART, C + 1], f32)
            consts_b = bass.AP(tensor=consts, offset=0, ap=[[0, PART], [1, C + 1]])
            nc.sync.dma_start(out=cvec[:], in_=consts_b)

            stats_ps = psum.tile([PX_PER_PART * C, PX_PER_PART * EW], f32)

            # gpsimd's big multiply shares an SBUF port with DVE: 2-port DVE
            # ops that overlap it stall ~10x. Gate each chunk's multiply on
            # the previous chunk's 2-port DVE epilogue so the multiply only
            # overlaps the (1-port, contention-immune) reduces.
            prev_small = []

            for ch in range(NCHUNK):
                a = big.tile([PART, FREE], f32, tag="a")
                nc.sync.dma_start(out=a[:], in_=acts[ch])
                lab = small.tile([PART, PX_PER_PART], f32, tag="lab")
                nc.sync.dma_start(out=lab[:], in_=labels[ch])

                e = ebuf.tile([PART, FREE], f32, tag="e")
                nc.scalar.activation(
                    out=e[:], in_=a[:], func=mybir.ActivationFunctionType.Exp
                )

                z = small.tile([PART, GF], f32, tag="z")
                nc.vector.tensor_reduce(
                    out=z[:],
                    in_=e[:].rearrange("p (g k) -> p g k", k=K),
                    axis=mybir.AxisListType.X,
                    op=mybir.AluOpType.add,
                )

                xe = xebuf.tile([PART, FREE], f32, tag="xe")
                mul_i = nc.gpsimd.tensor_mul(xe[:], a[:], e[:])
                for dep in prev_small:
                    add_dep_helper(mul_i.ins, dep.ins, reason="avoid DVE port clash")

                u = small.tile([PART, GF], f32, tag="u")
                nc.vector.tensor_reduce(
                    out=u[:],
                    in_=xe[:].rearrange("p (g k) -> p g k", k=K),
                    axis=mybir.AxisListType.X,
                    op=mybir.AluOpType.add,
                )

                logz = small.tile([PART, GF], f32, tag="logz")
                nc.scalar.activation(
                    out=logz[:], in_=z[:], func=mybir.ActivationFunctionType.Ln
                )
                rz = small.tile([PART, GF], f32, tag="rz")
                nc.scalar.activation(
                    out=rz[:],
                    in_=logz[:],
                    func=mybir.ActivationFunctionType.Exp,
                    scale=-1.0,
                )

                meanx = small.tile([PART, GF], f32, tag="meanx")
                mx_i = nc.vector.tensor_mul(meanx[:], u[:], rz[:])

                # ent tile [128, j=8, 33]: cols 0..31 = logZ - meanx, col 32 = 1.0
                ent = small.tile([PART, PX_PER_PART, EW], f32, tag="ent")
                ent_i = nc.vector.scalar_tensor_tensor(
                    out=ent[:, :, 0:G],
                    in0=meanx[:].rearrange("p (j g) -> p j g", g=G),
                    scalar=-1.0,
                    in1=logz[:].rearrange("p (j g) -> p j g", g=G),
                    op0=mybir.AluOpType.mult,
                    op1=mybir.AluOpType.add,
                )
                nc.vector.memset(ent[:, :, G : G + 1], 1.0)

                # mask [128, j=8, c=8] = (label[j] == c+1)
                mask = small.tile([PART, PX_PER_PART, C], f32, tag="mask")
                lab_ap = lab[:]
                lab_b = bass.AP(
                    tensor=lab_ap.tensor,
                    offset=lab_ap.offset,
                    ap=[lab_ap.ap[0], lab_ap.ap[1], [0, C]],
                )
                iota_ap = cvec[:, 0:C]
                iota_b = bass.AP(
                    tensor=iota_ap.tensor,
                    offset=iota_ap.offset,
                    ap=[iota_ap.ap[0], [0, PX_PER_PART], iota_ap.ap[1]],
                )
                mask_i = nc.vector.tensor_tensor(
                    mask[:], lab_b, iota_b, mybir.AluOpType.is_equal
                )
                prev_small = [mx_i, ent_i, mask_i]

                # stats[(j,c), (j',ew)] += mask^T @ [ent|1]
                nc.tensor.matmul(
                    out=stats_ps[:],
                    lhsT=mask[:].rearrange("p j c -> p (j c)"),
                    rhs=ent[:].rearrange("p j e -> p (j e)"),
                    start=(ch == 0),
                    stop=(ch == NCHUNK - 1),
                )

            stats_sb = singles.tile([PX_PER_PART * C, PX_PER_PART * EW], f32)
            nc.vector.tensor_copy(out=stats_sb[:], in_=stats_ps[:])
            nc.sync.dma_start(out=stats_out, in_=stats_sb[:])

    nc.compile()
    _CACHE["nc"] = nc
    return nc


def _prep_inputs(prototype_activations, target_labels, proto_idx):
    acts = np.asarray(prototype_activations, dtype=np.float32)
    labels = np.asarray(target_labels)
    pidx = np.asarray(proto_idx)

    expected = np.arange(S * C * K, dtype=np.int64).reshape(S, C, K)
    if not np.array_equal(pidx.astype(np.int64), expected):
        # general (slow) fallback: permute proto columns on host
        acts = np.ascontiguousarray(acts[..., pidx.reshape(-1)])

    labels_f = labels.astype(np.float32)
    consts = np.concatenate(
        [np.arange(1, C + 1, dtype=np.float32), np.ones(1, dtype=np.float32)]
    )

    in_maps = []
    for b in range(B):
        in_maps.append(
            {
                "acts": np.ascontiguousarray(acts[b]).reshape(NCHUNK, PART, FREE),
                "labels": np.ascontiguousarray(labels_f[b]).reshape(
                    NCHUNK, PART, PX_PER_PART
                ),
                "consts": consts,
            }
        )
    return in_maps


def _combine(stats_list):
    """stats_list: per-core [64, 264] arrays -> final scalar, in float32."""
    num = np.zeros((B, S, C), dtype=np.float32)
    cnt = np.zeros((B, C), dtype=np.float32)
    for b, st in enumerate(stats_list):
        st = st.reshape(PX_PER_PART, C, PX_PER_PART, EW)  # [j, c, j', e]
        diag = st[np.arange(PX_PER_PART), :, np.arange(PX_PER_PART), :]  # [j, c, e]
        cs = diag.sum(axis=0)  # [c, e]
        # e index: s*C + c' for ent cols, EW-1 = count
        ent_cols = cs[:, : S * C].reshape(C, S, C)  # [c, s, c']
        num[b] = ent_cols[np.arange(C), :, np.arange(C)].T  # [s, c]
        cnt[b] = cs[:, S * C]
    num /= np.float32(np.log(K))
    present = cnt > 0
    mean_ent = num / np.maximum(cnt, 1.0)[:, None, :]
    n_entries = np.float32(present.sum() * S)
    total = np.float32((mean_ent * present[:, None, :]).sum(dtype=np.float64))
    if n_entries > 0:
        out = np.float32(total / max(n_entries, np.float32(1.0)))
    else:
        out = np.float32(0.0)
    return out


def kernel(prototype_activations, target_labels, proto_idx, _trace=False, _tmpdir=None):
    nc = _build()
    in_maps = _prep_inputs(prototype_activations, target_labels, proto_idx)
    res = run_bass_kernel_spmd(
        nc, in_maps, list(range(NCORES)), trace=_trace, tmpdir=_tmpdir
    )
    stats_list = [res.results[i]["stats"] for i in range(NCORES)]
    out = _combine(stats_list)
    if _trace:
        return out, res
    return out
